# revision 4
# baseline (speedup 1.0000x reference)
import os
import sys

sys.path.insert(0, "/opt/trn_rl_repo")

import numpy as np
import ml_dtypes

BF16 = ml_dtypes.bfloat16
N_CORES = 8
B = 32          # batch per core
H = 256
V = 600
KA = 101        # 100 wv + 1 ones row (bias fold)
T_FULL = 255    # decoder steps

_CACHE = {}


def _build(n_steps):
    import concourse.bacc as bacc
    import concourse.tile as tile
    from concourse import mybir

    bf = mybir.dt.bfloat16
    f32 = mybir.dt.float32
    Sig = mybir.ActivationFunctionType.Sigmoid
    Tanh = mybir.ActivationFunctionType.Tanh

    nc = bacc.Bacc("TRN2", target_bir_lowering=False)

    capsT = nc.dram_tensor("capsT", [KA, n_steps, B], bf, kind="ExternalInput")
    wfold = nc.dram_tensor("wfold", [KA, 1024], bf, kind="ExternalInput")
    whht = nc.dram_tensor("whht", [128, 2, 1024], bf, kind="ExternalInput")
    woutt = nc.dram_tensor("woutt", [128, 2, V], bf, kind="ExternalInput")
    h0T = nc.dram_tensor("h0T", [128, 2, B], bf, kind="ExternalInput")
    c0 = nc.dram_tensor("c0", [B, H], f32, kind="ExternalInput")
    ident = nc.dram_tensor("ident", [B, B], f32, kind="ExternalInput")
    pred = nc.dram_tensor("pred", [n_steps, B, V], f32, kind="ExternalOutput")

    with tile.TileContext(nc) as tc:
        with (
            tc.tile_pool(name="const", bufs=1) as cp,
            tc.tile_pool(name="state", bufs=2) as sp,
            tc.tile_pool(name="acts", bufs=2) as apool,
            tc.tile_pool(name="psum", bufs=2, space="PSUM") as pp,
        ):
            capsT_sb = cp.tile([KA, n_steps, B], bf)
            nc.sync.dma_start(out=capsT_sb, in_=capsT[:])
            wfold_sb = cp.tile([KA, 1024], bf)
            nc.sync.dma_start(out=wfold_sb, in_=wfold[:])
            whht_sb = cp.tile([128, 2, 1024], bf)
            nc.sync.dma_start(out=whht_sb, in_=whht[:])
            woutt_sb = cp.tile([128, 2, V], bf)
            nc.sync.dma_start(out=woutt_sb, in_=woutt[:])
            ident_sb = cp.tile([B, B], f32)
            nc.sync.dma_start(out=ident_sb, in_=ident[:])
            # hT_all slot t holds h_{t-1}^T (slot 0 = h0); step t writes slot t+1
            hT_all = cp.tile([128, 2, n_steps + 1, B], bf)
            nc.sync.dma_start(out=hT_all[:, :, 0, :], in_=h0T[:])

            c_prev = sp.tile([B, H], f32, tag="c")
            nc.sync.dma_start(out=c_prev, in_=c0[:])

            for t in range(n_steps):
                gp = pp.tile([B, 1024], f32, tag="gates")
                lhs_x = capsT_sb[:, t, :]
                for hf in range(2):
                    sl = slice(hf * 512, (hf + 1) * 512)
                    nc.tensor.matmul(gp[:, sl], lhs_x, wfold_sb[:, sl],
                                     start=True, stop=False)
                    nc.tensor.matmul(gp[:, sl], hT_all[:, 0, t, :],
                                     whht_sb[:, 0, sl], start=False, stop=False)
                    nc.tensor.matmul(gp[:, sl], hT_all[:, 1, t, :],
                                     whht_sb[:, 1, sl], start=False, stop=True)
                # gate order (host-permuted): [i | f | o | g]
                act_s = apool.tile([B, 768], f32, tag="s")
                nc.scalar.activation(out=act_s, in_=gp[:, 0:768], func=Sig)
                act_g = apool.tile([B, 256], f32, tag="g")
                nc.scalar.activation(out=act_g, in_=gp[:, 768:1024], func=Tanh)

                ig = apool.tile([B, 256], f32, tag="ig")
                nc.vector.tensor_mul(ig, act_s[:, 0:256], act_g)
                fc = apool.tile([B, 256], f32, tag="fc")
                nc.vector.tensor_mul(fc, act_s[:, 256:512], c_prev)
                c_new = sp.tile([B, H], f32, tag="c")
                nc.vector.tensor_add(c_new, ig, fc)
                tch = apool.tile([B, 256], f32, tag="tc")
                nc.scalar.activation(out=tch, in_=c_new, func=Tanh)
                h_new = apool.tile([B, H], f32, tag="h")
                nc.vector.tensor_mul(h_new, act_s[:, 512:768], tch)

                hp = pp.tile([128, 2, B], f32, tag="hT")
                for kc in range(2):
                    nc.tensor.transpose(hp[:, kc, :],
                                        h_new[:, kc * 128:(kc + 1) * 128],
                                        ident_sb)
                nc.vector.tensor_copy(out=hT_all[:, :, t + 1, :], in_=hp)
                c_prev = c_new

            # phase 2: logits = H_all @ W_out.T, 4 timesteps per M-tile
            for t0 in range(0, n_steps, 4):
                tcnt = min(4, n_steps - t0)
                M = tcnt * B
                lg_sb = apool.tile([128, V], f32, tag="lg_sb")
                for n0, n1 in ((0, 300), (300, 600)):
                    lg = pp.tile([128, 300], f32, tag="lg")
                    for kc in range(2):
                        lhsT = hT_all[:, kc, t0 + 1:t0 + 1 + tcnt, :].rearrange(
                            "p t b -> p (t b)")
                        nc.tensor.matmul(lg[:M, :], lhsT,
                                         woutt_sb[:, kc, n0:n1],
                                         start=(kc == 0), stop=(kc == 1))
                    nc.vector.tensor_copy(out=lg_sb[:M, n0:n1], in_=lg[:M, :])
                nc.sync.dma_start(
                    out=pred[t0:t0 + tcnt].rearrange("t b v -> (t b) v"),
                    in_=lg_sb[:M, :])

    nc.compile()
    return nc


def _get_nc(n_steps):
    if n_steps not in _CACHE:
        _CACHE[n_steps] = _build(n_steps)
    return _CACHE[n_steps]


def _prep(encoder_out, encoded_captions, w_ih, w_hh, b_ih, b_hh,
          W_out, b_out, W_emb, b_emb, W_h, b_h, W_c, b_c,
          caption_lengths, n_steps):
    cl = np.asarray(caption_lengths)
    sort_idx = np.argsort(-cl, kind="stable")
    lengths = cl[sort_idx]
    enc = np.asarray(encoder_out)[sort_idx]
    caps = np.asarray(encoded_captions)[sort_idx]
    dec_len = lengths - 1

    perm = np.r_[0:512, 768:1024, 512:768]          # [i f g o] -> [i f o g]
    w_ih_p = w_ih[perm]
    w_hh_p = w_hh[perm]
    b_p = (b_ih + b_hh)[perm]

    Wfold = (W_emb.T @ w_ih_p.T).astype(np.float32)        # [100, 1024]
    bfold = (b_emb @ w_ih_p.T + b_p).astype(np.float32)    # [1024]
    wfold_aug = np.concatenate([Wfold, bfold[None, :]], 0).astype(BF16)

    whhT = w_hh_p.T.astype(np.float32)                     # [256, 1024]
    whht_in = np.stack([whhT[0:128], whhT[128:256]], axis=1).astype(BF16)
    WoutT = W_out.T.astype(np.float32)                     # [256, 600]
    woutt_in = np.stack([WoutT[0:128], WoutT[128:256]], axis=1).astype(BF16)

    h0 = (enc @ W_h.T + b_h).astype(np.float32)            # [256, 256]
    c0_full = (enc @ W_c.T + b_c).astype(np.float32)

    in_maps = []
    for i in range(N_CORES):
        bs = slice(i * B, (i + 1) * B)
        caps_c = caps[bs, :n_steps, :]                     # [B, T, 100]
        arr = np.ones((KA, n_steps, B), np.float32)
        arr[:100] = caps_c.transpose(2, 1, 0)
        h0cT = h0[bs].T                                    # [256, B]
        h0T_in = np.stack([h0cT[0:128], h0cT[128:256]], axis=1)  # [128,2,B]
        in_maps.append({
            "capsT": arr.astype(BF16),
            "wfold": wfold_aug,
            "whht": whht_in,
            "woutt": woutt_in,
            "h0T": h0T_in.astype(BF16),
            "c0": np.ascontiguousarray(c0_full[bs]),
            "ident": np.eye(B, dtype=np.float32),
        })
    return in_maps, caps, dec_len, sort_idx


def kernel(encoder_out, encoded_captions, w_ih, w_hh, b_ih, b_hh,
           W_out, b_out, W_emb, b_emb, W_h, b_h, W_c, b_c,
           caption_lengths, n_steps=T_FULL, _want_results=False):
    from concourse.bass_utils import run_bass_kernel_spmd

    in_maps, caps, dec_len, sort_idx = _prep(
        encoder_out, encoded_captions, w_ih, w_hh, b_ih, b_hh,
        W_out, b_out, W_emb, b_emb, W_h, b_h, W_c, b_c,
        caption_lengths, n_steps)

    nc = _get_nc(n_steps)
    trace = bool(int(os.environ.get("KERNEL_TRACE", "0")))
    import time as _time
    _t0 = _time.time()
    try:
        res = run_bass_kernel_spmd(nc, in_maps, core_ids=list(range(N_CORES)),
                                   trace=trace)
    except ModuleNotFoundError:
        res = run_bass_kernel_spmd(nc, in_maps, core_ids=list(range(N_CORES)))
    global LAST_EXEC_WALL_S
    LAST_EXEC_WALL_S = _time.time() - _t0

    preds = np.concatenate(
        [res.results[i]["pred"].transpose(1, 0, 2) for i in range(N_CORES)],
        axis=0)                                            # [256, T, 600]
    preds = preds + b_out.astype(np.float32)[None, None, :]
    mask = np.arange(n_steps)[None, :] >= dec_len[:, None]
    preds[mask] = -1.0
    preds = preds.astype(np.float32)

    if caption_lengths.dtype == np.int32:
        dec_len = dec_len.astype(np.int32)
        sort_idx = sort_idx.astype(np.int32)
    out = (preds, caps.astype(np.float32), dec_len, sort_idx)
    if _want_results:
        return out, res
    return out
